# revision 6
# baseline (speedup 1.0000x reference)
"""Causal multi-head attention on 8 Trainium2 NeuronCores.

Problem: B=2, S=4096, D_MODEL=768, H=12, D_HEAD=64, fp32 I/O.

Sharding: (batch, head-group) -> core.  Cores 0-3 take batch 0, cores 4-7
take batch 1; each core computes 3 of the 12 heads for its batch and emits a
partial output [S, D_MODEL] (its heads' contribution to the W_O contraction).
The host sums the 4 partials per batch (the "all-reduce" of the hint, done on
host since full I/O passes through host anyway) and adds b_O.

Per-core device program (all matmul compute in bf16, fp32 PSUM accumulation):
  1. QT/KT/VT[z, t] = W.T @ xT per head (weights stationary, xT moving)
  2. V[t, z] via PE transpose of VT; a ones-column is appended (for softmax
     row-sums)
  3. Flash attention per head over q-windows of 1024, scores kept [k, q] so
     exp(ACT) output PT feeds the AV matmul directly; Z accumulates in PSUM
     as [65 x 1024] with row 64 = sum of P (softmax denominator)
  4. Denominator r is PE-transposed to partitions, reciprocal'd on DVE,
     transposed back, broadcast across partitions with a rank-1 matmul, and
     Z is normalized with one DVE multiply
  5. Output projection accumulates all 3 heads into PSUM [t, 768]
"""

import numpy as np
import ml_dtypes

B, S, DM, H, DH = 2, 4096, 768, 12, 64
NCORES = 8
GROUPS = 4                  # head-groups per batch
HPC = H // GROUPS           # heads per core = 3
P = 128
QSUP = 1024                 # q-window per flash step
QCH = 512                   # psum bank width (fp32)

_BF = ml_dtypes.bfloat16

_cache = {}


def _build(seq_len, use_biases):
    import concourse.bacc as bacc
    import concourse.mybir as mybir
    import concourse.tile as tile

    f32 = mybir.dt.float32
    f32r = mybir.dt.float32r
    bf16 = mybir.dt.bfloat16
    Exp = mybir.ActivationFunctionType.Exp
    mult = mybir.AluOpType.mult

    SQ = seq_len
    qsup = min(QSUP, SQ)
    n_qs = SQ // qsup            # flash q-windows
    n_kt = SQ // P               # k tiles
    n_tt = SQ // P               # output row tiles
    n_ch = SQ // QCH             # 512-wide chunks
    DSL = DM // P                # 6 contraction slices for the projections
    kt_per_qs = qsup // P

    nc = bacc.Bacc(None, target_bir_lowering=False)

    xT = nc.declare_dram_parameter("xT", [DM, SQ], bf16, isOutput=False)
    wq = nc.declare_dram_parameter("wq", [DM, HPC * DH], bf16, isOutput=False)
    wk = nc.declare_dram_parameter("wk", [DM, HPC * DH], bf16, isOutput=False)
    wv = nc.declare_dram_parameter("wv", [DM, HPC * DH], bf16, isOutput=False)
    wo = nc.declare_dram_parameter("wo", [DH, HPC * DM], bf16, isOutput=False)
    trimask = nc.declare_dram_parameter("trimask", [P, P], bf16, isOutput=False)
    ident_f = nc.declare_dram_parameter("ident_f", [P, P], f32, isOutput=False)
    ident_b = nc.declare_dram_parameter("ident_b", [P, P], bf16, isOutput=False)
    ones_z = nc.declare_dram_parameter("ones_z", [1, DH], f32r, isOutput=False)
    if use_biases:
        bq = nc.declare_dram_parameter("bq", [DH, HPC], f32, isOutput=False)
        bk = nc.declare_dram_parameter("bk", [DH, HPC], f32, isOutput=False)
        bv = nc.declare_dram_parameter("bv", [DH, HPC], f32, isOutput=False)
    out = nc.declare_dram_parameter("out", [SQ, DM], f32, isOutput=True)

    with tile.TileContext(nc) as tc:
        with (
            tc.tile_pool(name="singles", bufs=1) as singles,
            tc.tile_pool(name="persist", bufs=1) as persist,
        ):
            # ---- constants / weights in SBUF ----
            xT_sb = singles.tile([P, DSL, SQ], bf16)
            for o in range(DSL):
                nc.sync.dma_start(xT_sb[:, o, :], xT[o * P:(o + 1) * P, :])
            w_sb = {}
            for name, drm in (("q", wq), ("k", wk), ("v", wv)):
                t = singles.tile([P, DSL, HPC * DH], bf16, tag=f"w{name}")
                nc.sync.dma_start(
                    t[:], drm.rearrange("(o p) c -> p o c", p=P))
                w_sb[name] = t
            wo_sb = singles.tile([DH, HPC, DM], bf16)
            nc.sync.dma_start(wo_sb[:], wo.rearrange("z (h d) -> z h d", h=HPC))
            tri_sb = singles.tile([P, P], bf16)
            nc.sync.dma_start(tri_sb[:], trimask[:])
            idf_sb = singles.tile([P, P], f32)
            nc.sync.dma_start(idf_sb[:], ident_f[:])
            idb_sb = singles.tile([P, P], bf16)
            nc.sync.dma_start(idb_sb[:], ident_b[:])
            ones_sb = singles.tile([1, DH], f32r)
            nc.sync.dma_start(ones_sb[:], ones_z[:])
            bias_sb = {}
            if use_biases:
                for name, drm in (("q", bq), ("k", bk), ("v", bv)):
                    t = singles.tile([DH, HPC], f32, tag=f"b{name}")
                    nc.sync.dma_start(t[:], drm[:])
                    bias_sb[name] = t

            # ---- persistent activations ----
            QT_sb = persist.tile([DH, HPC, SQ], bf16, tag="QT")
            KT_sb = persist.tile([DH, HPC, SQ], bf16, tag="KT")
            V_sb = persist.tile([P, HPC, n_kt, DH + 1], bf16, tag="V")
            Zn_sb = persist.tile([DH, HPC, n_qs, qsup], bf16, tag="Zn")

            # ================= QKV projections =================
            with (
                tc.tile_pool(name="qkv_ps", bufs=3, space="PSUM") as qkv_ps,
                tc.tile_pool(name="vt_ps", bufs=2, space="PSUM") as vt_ps,
                tc.tile_pool(name="vt_sb", bufs=2) as vt_pool,
            ):
                for tname, dest in (("q", QT_sb), ("k", KT_sb), ("v", None)):
                    for h in range(HPC):
                        for c in range(n_ch):
                            ps = qkv_ps.tile([DH, QCH], f32, tag="proj")
                            for o in range(DSL):
                                nc.tensor.matmul(
                                    ps[:],
                                    lhsT=w_sb[tname][:, o, h * DH:(h + 1) * DH],
                                    rhs=xT_sb[:, o, c * QCH:(c + 1) * QCH],
                                    start=(o == 0), stop=(o == DSL - 1),
                                )
                            if tname != "v":
                                dst = dest[:, h, c * QCH:(c + 1) * QCH]
                                if use_biases:
                                    nc.scalar.add(dst, ps[:],
                                                  bias_sb[tname][:, h:h + 1])
                                else:
                                    nc.vector.tensor_copy(dst, ps[:])
                            else:
                                vt = vt_pool.tile([DH, QCH], bf16, tag="vt")
                                if use_biases:
                                    nc.scalar.add(vt[:], ps[:],
                                                  bias_sb["v"][:, h:h + 1])
                                else:
                                    nc.vector.tensor_copy(vt[:], ps[:])
                                for j in range(QCH // P):
                                    kt = c * (QCH // P) + j
                                    vp = vt_ps.tile([P, DH], bf16, tag="vtp")
                                    nc.tensor.transpose(
                                        vp[:], vt[:, j * P:(j + 1) * P],
                                        idb_sb[:DH, :DH])
                                    nc.vector.tensor_copy(
                                        V_sb[:, h, kt, 0:DH], vp[:])
                nc.vector.memset(V_sb[:, :, :, DH:DH + 1], 1.0)

            # ================= flash attention =================
            with (
                tc.tile_pool(name="s_ps", bufs=2, space="PSUM") as s_ps,
                tc.tile_pool(name="z_ps", bufs=1, space="PSUM") as z_ps,
                tc.tile_pool(name="n_ps", bufs=1, space="PSUM") as n_ps,
                tc.tile_pool(name="pt_sb", bufs=3) as pt_pool,
                tc.tile_pool(name="nrm_sb", bufs=2) as nrm_pool,
            ):
                for h in range(HPC):
                    for qs in range(n_qs):
                        q0 = qs * qsup
                        zacc = z_ps.tile([DH + 1, qsup], f32, tag="zacc")
                        nk = kt_per_qs * qs + kt_per_qs
                        for ki in range(nk):
                            vs = max(0, P * ki - q0)
                            ssc = s_ps.tile([P, qsup], f32, tag="S")
                            for half in range(qsup // QCH):
                                lo = max(vs, half * QCH)
                                hi = (half + 1) * QCH
                                if lo < hi:
                                    nc.tensor.matmul(
                                        ssc[:, lo:hi],
                                        lhsT=KT_sb[:, h, ki * P:(ki + 1) * P],
                                        rhs=QT_sb[:, h, q0 + lo:q0 + hi],
                                        start=True, stop=True,
                                    )
                            pt = pt_pool.tile([P, qsup], bf16, tag="PT")
                            nc.scalar.activation(
                                pt[:, vs:], ssc[:, vs:], Exp, scale=0.125)
                            if ki >= kt_per_qs * qs:  # diagonal tile
                                nc.vector.tensor_tensor(
                                    pt[:, vs:vs + P], pt[:, vs:vs + P],
                                    tri_sb[:], mult)
                            for half in range(qsup // QCH):
                                lo = max(vs, half * QCH)
                                hi = (half + 1) * QCH
                                if lo < hi:
                                    nc.tensor.matmul(
                                        zacc[:, lo:hi],
                                        lhsT=V_sb[:, h, ki, :],
                                        rhs=pt[:, lo:hi],
                                        start=(ki == 0), stop=(ki == nk - 1),
                                    )
                        # ---- normalization ----
                        r_sb = nrm_pool.tile([1, qsup], f32, tag="r")
                        nc.vector.tensor_copy(r_sb[:], zacc[DH:DH + 1, :])
                        rT = n_ps.tile([P, qsup // P], f32, tag="norm")
                        for j in range(qsup // P):
                            nc.tensor.transpose(
                                rT[:, j:j + 1], r_sb[0:1, j * P:(j + 1) * P],
                                idf_sb[0:1, 0:1])
                        rrT = nrm_pool.tile([P, qsup // P], f32, tag="rrT")
                        nc.vector.reciprocal(rrT[:], rT[:])
                        rrp = n_ps.tile([1, qsup], f32, tag="norm")
                        for j in range(qsup // P):
                            nc.tensor.transpose(
                                rrp[0:1, j * P:(j + 1) * P], rrT[:, j:j + 1],
                                idf_sb[:, :])
                        rr_sb = nrm_pool.tile([1, qsup], f32r, tag="rr")
                        nc.vector.tensor_copy(rr_sb[:], rrp[:])
                        rrb = s_ps.tile([P, qsup], f32, tag="S")
                        for half in range(qsup // QCH):
                            sl = slice(half * QCH, (half + 1) * QCH)
                            nc.tensor.matmul(
                                rrb[:DH, sl],
                                lhsT=ones_sb[:],
                                rhs=rr_sb[:, sl],
                                start=True, stop=True,
                            )
                        zsb = nrm_pool.tile([DH, qsup], bf16, tag="zsb")
                        nc.vector.tensor_copy(zsb[:], zacc[0:DH, :])
                        nc.vector.tensor_tensor(
                            Zn_sb[:, h, qs, :], zsb[:], rrb[:DH, :], mult)

            # ================= output projection =================
            with (
                tc.tile_pool(name="o_ps", bufs=2, space="PSUM") as o_ps,
                tc.tile_pool(name="o_sb", bufs=3) as o_pool,
            ):
                for tt in range(n_tt):
                    qs, off = divmod(tt * P, qsup)
                    po = o_ps.tile([P, DM], f32, tag="po")
                    for h in range(HPC):
                        lhsT = Zn_sb[:, h, qs, off:off + P]
                        nc.tensor.matmul(po[:, 0:QCH], lhsT,
                                         rhs=wo_sb[:, h, 0:QCH],
                                         start=(h == 0), stop=(h == HPC - 1))
                        nc.tensor.matmul(po[:, QCH:DM], lhsT,
                                         rhs=wo_sb[:, h, QCH:DM],
                                         start=(h == 0), stop=(h == HPC - 1))
                    osb = o_pool.tile([P, DM], f32, tag="osb")
                    nc.scalar.copy(osb[:], po[:])
                    nc.sync.dma_start(out[tt * P:(tt + 1) * P, :], osb[:])

    nc.compile()
    return nc


def _prep_inputs(inputs, seq_len, use_biases):
    x = np.asarray(inputs["normalized_resid_pre"], dtype=np.float32)
    WQ = np.asarray(inputs["W_Q"], dtype=np.float32)
    WK = np.asarray(inputs["W_K"], dtype=np.float32)
    WV = np.asarray(inputs["W_V"], dtype=np.float32)
    WO = np.asarray(inputs["W_O"], dtype=np.float32)

    tri = np.triu(np.ones((P, P), np.float32)).astype(_BF)  # keep j >= p
    idf = np.eye(P, dtype=np.float32)
    idb = np.eye(P, dtype=np.float32).astype(_BF)
    onz = np.ones((1, DH), np.float32)

    in_maps = []
    for c in range(NCORES):
        b, g = divmod(c, GROUPS)
        hs = slice(g * HPC, (g + 1) * HPC)
        m = {
            "xT": np.ascontiguousarray(x[b, :seq_len].T).astype(_BF),
            "wq": np.ascontiguousarray(
                WQ[hs].transpose(1, 0, 2).reshape(DM, HPC * DH)).astype(_BF),
            "wk": np.ascontiguousarray(
                WK[hs].transpose(1, 0, 2).reshape(DM, HPC * DH)).astype(_BF),
            "wv": np.ascontiguousarray(
                WV[hs].transpose(1, 0, 2).reshape(DM, HPC * DH)).astype(_BF),
            "wo": np.ascontiguousarray(
                WO[hs].transpose(1, 0, 2).reshape(DH, HPC * DM)).astype(_BF),
            "trimask": tri,
            "ident_f": idf,
            "ident_b": idb,
            "ones_z": onz,
        }
        if use_biases:
            m["bq"] = np.ascontiguousarray(
                np.asarray(inputs["b_Q"], np.float32)[hs].T)
            m["bk"] = np.ascontiguousarray(
                np.asarray(inputs["b_K"], np.float32)[hs].T)
            m["bv"] = np.ascontiguousarray(
                np.asarray(inputs["b_V"], np.float32)[hs].T)
        in_maps.append(m)
    return in_maps


TRACE = False          # test.py can flip this to get exec_time_ns
last_result = None     # BassKernelResults of the most recent run


def kernel(seq_len=S, **inputs):
    global last_result
    from concourse.bass_utils import run_bass_kernel_spmd

    use_biases = any(
        np.any(np.asarray(inputs[k]) != 0) for k in ("b_Q", "b_K", "b_V"))

    key = (seq_len, use_biases)
    if key not in _cache:
        _cache[key] = _build(seq_len, use_biases)
    nc = _cache[key]

    in_maps = _prep_inputs(inputs, seq_len, use_biases)
    res = run_bass_kernel_spmd(nc, in_maps, core_ids=list(range(NCORES)),
                               trace=TRACE)
    last_result = res

    b_O = np.asarray(inputs["b_O"], dtype=np.float32)
    out = np.zeros((B, seq_len, DM), np.float32)
    for c in range(NCORES):
        b = c // GROUPS
        out[b] += np.asarray(res.results[c]["out"], dtype=np.float32)
    out += b_O[None, None, :]
    return out


# revision 15
# speedup vs baseline: 1.4186x; 1.4186x over previous
"""Causal multi-head attention on 8 Trainium2 NeuronCores.

Problem: B=2, S=4096, D_MODEL=768, H=12, D_HEAD=64, fp32 I/O.

Sharding: (batch, head-group) -> core.  Cores 0-3 take batch 0, cores 4-7
take batch 1; each core computes 3 of the 12 heads for its batch and emits a
partial output [S, D_MODEL] (its heads' contribution to the W_O contraction).
The host sums the 4 partials per batch and adds b_O.

Per-core device program (matmul compute in bf16, fp32 PSUM accumulation):
  1. QT/KT[z, t] = W.T @ xT; heads 0,1 packed on partition halves (0-63 /
     64-127) so their scores matmuls run concurrently in different PE row
     groups; head 2 separate.  VT computed per head, PE-transposed to V[t, z]
     with a ones column appended (softmax row sums).
  2. Flash attention, scores [k, q] so exp output PT feeds the AV matmul
     directly; Z accumulates in PSUM [65 x W] with row 64 = sum(P).
  3. Softmax normalization runs entirely on DVE + GPSIMD (32x32 block
     transposes + strided reciprocal + partition_broadcast) so the PE FIFO
     stays a pure matmul stream and HAM keeps the PE at 2.4 GHz.
  4. Output projection accumulates all 3 heads into PSUM [t, 768].
"""

import numpy as np
import ml_dtypes

B, S, DM, H, DH = 2, 4096, 768, 12, 64
NCORES = 8
GROUPS = 4                  # head-groups per batch
HPC = H // GROUPS           # heads per core = 3
P = 128
QCH = 512                   # psum bank width (fp32)

_BF = ml_dtypes.bfloat16

_cache = {}


def _build(seq_len, use_biases):
    import concourse.bacc as bacc
    import concourse.mybir as mybir
    import concourse.tile as tile

    f32 = mybir.dt.float32
    bf16 = mybir.dt.bfloat16
    Exp = mybir.ActivationFunctionType.Exp
    mult = mybir.AluOpType.mult

    SQ = seq_len
    n_kt = SQ // P               # k tiles
    n_tt = SQ // P               # output row tiles
    n_ch = SQ // QCH             # 512-wide chunks
    DSL = DM // P                # contraction slices for the projections

    nc = bacc.Bacc(None, target_bir_lowering=False)

    xT = nc.declare_dram_parameter("xT", [DM, SQ], bf16, isOutput=False)
    wq = nc.declare_dram_parameter("wq", [DM, HPC * DH], bf16, isOutput=False)
    wk = nc.declare_dram_parameter("wk", [DM, HPC * DH], bf16, isOutput=False)
    wv = nc.declare_dram_parameter("wv", [DM, HPC * DH], bf16, isOutput=False)
    wo = nc.declare_dram_parameter("wo", [DH, HPC * DM], bf16, isOutput=False)
    trimask = nc.declare_dram_parameter("trimask", [P, P], bf16, isOutput=False)
    ident_b = nc.declare_dram_parameter("ident_b", [P, P], bf16, isOutput=False)
    if use_biases:
        bqkv_p = nc.declare_dram_parameter("bqkv_p", [P, 2], f32, isOutput=False)
        bqkv_s = nc.declare_dram_parameter("bqkv_s", [DH, 2], f32, isOutput=False)
        bv_all = nc.declare_dram_parameter("bv_all", [DH, HPC], f32,
                                           isOutput=False)
    out = nc.declare_dram_parameter("out", [SQ, DM], f32, isOutput=True)

    with tile.TileContext(nc) as tc:
        with (
            tc.tile_pool(name="singles", bufs=1) as singles,
            tc.tile_pool(name="persist", bufs=1) as persist,
        ):
            # ---- constants / weights ----
            w_sb = {}
            for name, drm in (("q", wq), ("k", wk), ("v", wv)):
                t = singles.tile([P, DSL, HPC * DH], bf16, tag=f"w{name}")
                nc.sync.dma_start(t[:], drm.rearrange("(o p) c -> p o c", p=P))
                w_sb[name] = t
            wo_sb = singles.tile([DH, HPC, DM], bf16)
            nc.sync.dma_start(wo_sb[:], wo.rearrange("z (h d) -> z h d", h=HPC))
            tri_sb = singles.tile([P, P], bf16)
            nc.sync.dma_start(tri_sb[:], trimask[:])
            idb_sb = singles.tile([P, P], bf16)
            nc.sync.dma_start(idb_sb[:], ident_b[:])
            bias_p = bias_s = bias_v = None
            if use_biases:
                bias_p = singles.tile([P, 2], f32, tag="bp")
                nc.sync.dma_start(bias_p[:], bqkv_p[:])
                bias_s = singles.tile([DH, 2], f32, tag="bs")
                nc.sync.dma_start(bias_s[:], bqkv_s[:])
                bias_v = singles.tile([DH, HPC], f32, tag="bv")
                nc.sync.dma_start(bias_v[:], bv_all[:])

            # ---- persistent activations ----
            QT2 = persist.tile([P, SQ], bf16, tag="QT2")   # heads 0,1 stacked
            KT2 = persist.tile([P, SQ], bf16, tag="KT2")
            QTs = persist.tile([DH, SQ], bf16, tag="QTs")  # head 2
            KTs = persist.tile([DH, SQ], bf16, tag="KTs")
            V_sb = persist.tile([P, HPC, n_kt, DH + 1], bf16, tag="V")
            Zn_sb = persist.tile([DH, HPC, SQ], bf16, tag="Zn")

            # ================= QKV projections =================
            with (
                tc.tile_pool(name="xT_pool", bufs=1) as xT_pool,
                tc.tile_pool(name="qkv_ps", bufs=3, space="PSUM") as qkv_ps,
                tc.tile_pool(name="vt_ps", bufs=3, space="PSUM") as vt_ps,
                tc.tile_pool(name="vt_sb", bufs=2) as vt_pool,
            ):
                xT_sb = xT_pool.tile([P, DSL, SQ], bf16)
                for o in range(DSL):
                    nc.sync.dma_start(xT_sb[:, o, :], xT[o * P:(o + 1) * P, :])

                def proj(tname, w_cols, dst, bias, c):
                    m = w_cols.stop - w_cols.start
                    ps = qkv_ps.tile([P, QCH], f32, tag="proj",
                                     name="proj_ps")[:m]
                    for o in range(DSL):
                        nc.tensor.matmul(
                            ps[:],
                            lhsT=w_sb[tname][:, o, w_cols],
                            rhs=xT_sb[:, o, c * QCH:(c + 1) * QCH],
                            start=(o == 0), stop=(o == DSL - 1),
                        )
                    if bias is not None:
                        nc.scalar.add(dst, ps[:], bias)
                    else:
                        nc.vector.tensor_copy(dst, ps[:])

                for tname, d2, ds in (("q", QT2, QTs), ("k", KT2, KTs)):
                    bi = {"q": 0, "k": 1}[tname]
                    for c in range(n_ch):
                        proj(tname, slice(0, P),
                             d2[:, c * QCH:(c + 1) * QCH],
                             bias_p[:, bi:bi + 1] if use_biases else None, c)
                    for c in range(n_ch):
                        proj(tname, slice(P, P + DH),
                             ds[:, c * QCH:(c + 1) * QCH],
                             bias_s[:, bi:bi + 1] if use_biases else None, c)
                # V: per head, then batched PE transposes
                for h in range(HPC):
                    vt_full = vt_pool.tile([DH, SQ], bf16, tag="vtf")
                    for c in range(n_ch):
                        proj("v", slice(h * DH, (h + 1) * DH),
                             vt_full[:, c * QCH:(c + 1) * QCH],
                             bias_v[:, h:h + 1] if use_biases else None, c)
                    for kt in range(n_kt):
                        vp = vt_ps.tile([P, DH], bf16, tag="vtp")
                        nc.tensor.transpose(
                            vp[:], vt_full[:, kt * P:(kt + 1) * P],
                            idb_sb[:DH, :DH])
                        nc.vector.tensor_copy(V_sb[:, h, kt, 0:DH], vp[:])
                nc.vector.memset(V_sb[:, :, :, DH:DH + 1], 1.0)

            # ---- softmax normalization chain: DVE + GPSIMD only ----
            def norm_chain(nrm_pool, zacc, h, q0, width):
                nb = width // 32
                r32 = nrm_pool.tile([32, width], f32, tag="r32")
                nc.vector.tensor_copy(r32[0:1, :], zacc[DH:DH + 1, :])
                rT = nrm_pool.tile([32, width], f32, tag="rT")
                nc.vector.transpose(rT[:], r32[:])
                rrT = nrm_pool.tile([32, width], f32, tag="rrT")
                nc.vector.reciprocal(
                    rrT.rearrange("p (j c) -> p j c", c=32)[:, :, 0],
                    rT.rearrange("p (j c) -> p j c", c=32)[:, :, 0])
                rr32 = nrm_pool.tile([32, width], f32, tag="rr32")
                nc.vector.transpose(rr32[:], rrT[:])
                rrb = nrm_pool.tile([DH, width], f32, tag="rrb")
                nc.gpsimd.partition_broadcast(rrb[:], rr32[0:1, :])
                zsb = nrm_pool.tile([DH, width], bf16, tag="zsb")
                nc.vector.tensor_copy(zsb[:], zacc[0:DH, :])
                nc.vector.tensor_tensor(
                    Zn_sb[:, h, q0:q0 + width], zsb[:], rrb[:], mult)

            # ================= flash: head pair (0,1) =================
            with (
                tc.tile_pool(name="s_ps", bufs=2, space="PSUM") as s_ps,
                tc.tile_pool(name="z_ps", bufs=4, space="PSUM") as z_ps,
                tc.tile_pool(name="pt_sb", bufs=3) as pt_pool,
                tc.tile_pool(name="nrm_sb", bufs=2) as nrm_pool,
            ):
                for qs in range(SQ // QCH):
                    q0 = qs * QCH
                    za = z_ps.tile([DH + 1, QCH], f32, tag="zacc")
                    zb = z_ps.tile([DH + 1, QCH], f32, tag="zacc")
                    nk = 4 * qs + 4
                    for ki in range(nk):
                        vs = max(0, P * ki - q0)
                        ssc = s_ps.tile([P, 2 * QCH], f32, tag="S")
                        nc.tensor.matmul(
                            ssc[:, vs:QCH],
                            lhsT=KT2[0:DH, ki * P:(ki + 1) * P],
                            rhs=QT2[0:DH, q0 + vs:q0 + QCH],
                            start=True, stop=True)
                        nc.tensor.matmul(
                            ssc[:, QCH + vs:2 * QCH],
                            lhsT=KT2[DH:P, ki * P:(ki + 1) * P],
                            rhs=QT2[DH:P, q0 + vs:q0 + QCH],
                            start=True, stop=True)
                        pt = pt_pool.tile([P, 2 * QCH], bf16, tag="PT")
                        nc.scalar.activation(
                            pt[:, vs:], ssc[:, vs:], Exp, scale=0.125)
                        if ki >= 4 * qs:  # diagonal tile: mask both heads
                            blk = pt.rearrange(
                                "p (c w) -> p c w", c=2)[:, :, vs:vs + P]
                            nc.vector.tensor_tensor(
                                blk, blk,
                                tri_sb[:, None, :].to_broadcast(blk.shape),
                                mult)
                        nc.tensor.matmul(
                            za[:, vs:QCH], lhsT=V_sb[:, 0, ki, :],
                            rhs=pt[:, vs:QCH],
                            start=(ki == 0), stop=(ki == nk - 1))
                        nc.tensor.matmul(
                            zb[:, vs:QCH], lhsT=V_sb[:, 1, ki, :],
                            rhs=pt[:, QCH + vs:2 * QCH],
                            start=(ki == 0), stop=(ki == nk - 1))
                    norm_chain(nrm_pool, za, 0, q0, QCH)
                    norm_chain(nrm_pool, zb, 1, q0, QCH)

            # ================= flash: head 2 =================
            QSUP = min(2 * QCH, SQ)
            with (
                tc.tile_pool(name="s2_ps", bufs=2, space="PSUM") as s2_ps,
                tc.tile_pool(name="z2_ps", bufs=2, space="PSUM") as z2_ps,
                tc.tile_pool(name="pt2_sb", bufs=3) as pt2_pool,
                tc.tile_pool(name="nrm2_sb", bufs=2) as nrm2_pool,
            ):
                kt_per_w = QSUP // P
                for qs in range(SQ // QSUP):
                    q0 = qs * QSUP
                    zacc = z2_ps.tile([DH + 1, QSUP], f32, tag="zacc2")
                    nk = kt_per_w * qs + kt_per_w
                    for ki in range(nk):
                        vs = max(0, P * ki - q0)
                        ssc = s2_ps.tile([P, QSUP], f32, tag="S2")
                        for half in range(QSUP // QCH):
                            lo = max(vs, half * QCH)
                            hi = (half + 1) * QCH
                            if lo < hi:
                                nc.tensor.matmul(
                                    ssc[:, lo:hi],
                                    lhsT=KTs[:, ki * P:(ki + 1) * P],
                                    rhs=QTs[:, q0 + lo:q0 + hi],
                                    start=True, stop=True)
                        pt = pt2_pool.tile([P, QSUP], bf16, tag="PT2")
                        nc.scalar.activation(
                            pt[:, vs:], ssc[:, vs:], Exp, scale=0.125)
                        if ki >= kt_per_w * qs:
                            nc.vector.tensor_tensor(
                                pt[:, vs:vs + P], pt[:, vs:vs + P],
                                tri_sb[:], mult)
                        for half in range(QSUP // QCH):
                            lo = max(vs, half * QCH)
                            hi = (half + 1) * QCH
                            if lo < hi:
                                nc.tensor.matmul(
                                    zacc[:, lo:hi], lhsT=V_sb[:, 2, ki, :],
                                    rhs=pt[:, lo:hi],
                                    start=(ki == 0), stop=(ki == nk - 1))
                    norm_chain(nrm2_pool, zacc, 2, q0, QSUP)

            # ================= output projection =================
            with (
                tc.tile_pool(name="o_ps", bufs=2, space="PSUM") as o_ps,
                tc.tile_pool(name="o_sb", bufs=3) as o_pool,
            ):
                for tt in range(n_tt):
                    po = o_ps.tile([P, DM], f32, tag="po")
                    for h in range(HPC):
                        lhsT = Zn_sb[:, h, tt * P:(tt + 1) * P]
                        nc.tensor.matmul(po[:, 0:QCH], lhsT,
                                         rhs=wo_sb[:, h, 0:QCH],
                                         start=(h == 0), stop=(h == HPC - 1))
                        nc.tensor.matmul(po[:, QCH:DM], lhsT,
                                         rhs=wo_sb[:, h, QCH:DM],
                                         start=(h == 0), stop=(h == HPC - 1))
                    osb = o_pool.tile([P, DM], f32, tag="osb")
                    nc.scalar.copy(osb[:], po[:])
                    nc.sync.dma_start(out[tt * P:(tt + 1) * P, :], osb[:])

    nc.compile()
    return nc


def _prep_inputs(inputs, seq_len, use_biases):
    x = np.asarray(inputs["normalized_resid_pre"], dtype=np.float32)
    WQ = np.asarray(inputs["W_Q"], dtype=np.float32)
    WK = np.asarray(inputs["W_K"], dtype=np.float32)
    WV = np.asarray(inputs["W_V"], dtype=np.float32)
    WO = np.asarray(inputs["W_O"], dtype=np.float32)

    tri = np.triu(np.ones((P, P), np.float32)).astype(_BF)  # keep j >= p
    idb = np.eye(P, dtype=np.float32).astype(_BF)

    in_maps = []
    for c in range(NCORES):
        b, g = divmod(c, GROUPS)
        hs = slice(g * HPC, (g + 1) * HPC)
        m = {
            "xT": np.ascontiguousarray(x[b, :seq_len].T).astype(_BF),
            "wq": np.ascontiguousarray(
                WQ[hs].transpose(1, 0, 2).reshape(DM, HPC * DH)).astype(_BF),
            "wk": np.ascontiguousarray(
                WK[hs].transpose(1, 0, 2).reshape(DM, HPC * DH)).astype(_BF),
            "wv": np.ascontiguousarray(
                WV[hs].transpose(1, 0, 2).reshape(DM, HPC * DH)).astype(_BF),
            "wo": np.ascontiguousarray(
                WO[hs].transpose(1, 0, 2).reshape(DH, HPC * DM)).astype(_BF),
            "trimask": tri,
            "ident_b": idb,
        }
        if use_biases:
            bq = np.asarray(inputs["b_Q"], np.float32)[hs]
            bk = np.asarray(inputs["b_K"], np.float32)[hs]
            bv = np.asarray(inputs["b_V"], np.float32)[hs]
            # pair layout: [128, 2] = heads {0,1} stacked, cols q/k
            m["bqkv_p"] = np.stack(
                [np.concatenate([bq[0], bq[1]]),
                 np.concatenate([bk[0], bk[1]])], axis=1)
            m["bqkv_s"] = np.stack([bq[2], bk[2]], axis=1)
            m["bv_all"] = np.ascontiguousarray(bv.T)
        in_maps.append(m)
    return in_maps


TRACE = False          # test.py can flip this to get exec_time_ns
last_result = None     # BassKernelResults of the most recent run


def kernel(seq_len=S, **inputs):
    global last_result
    from concourse.bass_utils import run_bass_kernel_spmd

    use_biases = any(
        np.any(np.asarray(inputs[k]) != 0) for k in ("b_Q", "b_K", "b_V"))

    key = (seq_len, use_biases)
    if key not in _cache:
        _cache[key] = _build(seq_len, use_biases)
    nc = _cache[key]

    in_maps = _prep_inputs(inputs, seq_len, use_biases)
    res = run_bass_kernel_spmd(nc, in_maps, core_ids=list(range(NCORES)),
                               trace=TRACE)
    last_result = res

    b_O = np.asarray(inputs["b_O"], dtype=np.float32)
    out = np.zeros((B, seq_len, DM), np.float32)
    for c in range(NCORES):
        b = c // GROUPS
        out[b] += np.asarray(res.results[c]["out"], dtype=np.float32)
    out += b_O[None, None, :]
    return out
